# revision 1
# baseline (speedup 1.0000x reference)
"""Trainium2 Bass kernel for DescartesExtension (order-2, with replacement).

out[b, k] = x[b, ii[k]] * x[b, jj[k]] with (ii, jj) = triu_indices(D), i.e.
the output row is the concatenation over i of x[b, i] * x[b, i:D].

Sharding: data-parallel over the batch dim — 1024 rows / 8 cores = 128 rows
per core, which is exactly one SBUF partition tile. Per core the kernel:
  1. loads its [128, 512] x shard into SBUF (one tiny DMA),
  2. for each i computes the segment x[:, i] * x[:, i:] with a per-partition
     broadcast multiply (VectorE tensor_scalar or ScalarE activation-Copy
     with a [128,1] scale operand), packing segments contiguously into
     SBUF chunks,
  3. DMAs each chunk to its slice of the output row via the SP HWDGE ring.

The problem is HBM-write bound (538 MB total output vs 2 MB input), so the
structure is built around keeping the 16 SDMA engines saturated:
  - chunk free-dim <= 16384 elements (64 KB rows): one descriptor per
    partition and ~26 GB/s per SDMA engine; longer rows get shattered into
    sub-KB descriptors (13 GB/s),
  - small ramp-up chunks (each with its own buffer) so the first DMA issues
    a few us in and ramp DMAs don't serialize on completion latency,
  - chunk processing order interleaves front chunks (few long segments,
    fast to compute) with back chunks (many short segments, slow to
    compute) so the DMA queue always has backlog and the last computed
    chunk is a fast one,
  - per-segment greedy split of the multiply work across VectorE and
    ScalarE using measured costs (DVE ~212+0.52*L ns, ACT ~371+0.84*L ns),
  - a dummy ScalarE activation up front so the one-time ACT table load
    (~2.7 us) overlaps the x load instead of gating the first chunks.
"""

import numpy as np

N_CORES = 8
B = 1024
D = 512
K = D * (D + 1) // 2  # 131328
BS = B // N_CORES  # 128 rows per core = one partition tile

RAMP_UP = [512, 4096]
# Chunks overshoot their target by up to one segment (<=512); 15872 keeps the
# final length <= 16384 elements so each partition row stays one descriptor.
STEADY_TARGET = 15872
STEADY_BUFS = 2

_CACHE = {}


def _segments():
    lengths = [D - i for i in range(D)]
    offs = [0]
    for ln in lengths:
        offs.append(offs[-1] + ln)
    return lengths, offs


def _chunks(lengths):
    """Segment-aligned chunks: ramp-up targets, then steady."""
    targets = list(RAMP_UP)
    chunks = []
    i = 0
    off = 0
    while i < D:
        target = targets.pop(0) if targets else STEADY_TARGET
        s = i
        clen = 0
        while i < D and clen < target:
            clen += lengths[i]
            i += 1
        chunks.append((s, i, off, clen))
        off += clen
    return chunks


def _issue_order(n_chunks, n_ramp):
    """Ramp chunks first, then alternate front/back steady chunks.

    Back chunks hold many short segments (compute-heavy, per-op overhead
    dominated); pairing each with a fast front chunk keeps aggregate chunk
    production ahead of the DMA drain everywhere in the stream.
    """
    order = list(range(n_ramp))
    front = n_ramp
    back = n_chunks - 1
    take_front = True
    while front <= back:
        if take_front:
            order.append(front)
            front += 1
        else:
            order.append(back)
            back -= 1
        take_front = not take_front
    return order


def _engine_split(lengths, chunks, order, n_ramp):
    """Greedy per-segment balance between VectorE and ScalarE in issue order.

    Measured on HW: DVE fp32 tensor_scalar ~= 212 + 0.522*L ns (two-port
    mode), ACT activation-Copy ~= 371 + 0.840*L ns. Ramp segments are pinned
    to VectorE so the ACT table load can't gate the first DMAs.
    """
    t_v = 0.0
    t_s = 0.0
    assign = {}
    for ci in order:
        s, e, _off0, _clen = chunks[ci]
        for i in range(s, e):
            ln = lengths[i]
            c_v = 212.0 + 0.522 * ln
            c_s = 371.0 + 0.840 * ln
            if ci == 0 or t_v + c_v <= t_s + c_s:
                assign[i] = "v"
                t_v += c_v
            else:
                assign[i] = "s"
                t_s += c_s
    return assign


def _build():
    if "nc" in _CACHE:
        return _CACHE["nc"]
    import concourse.tile as tile
    from concourse import bacc, mybir

    nc = bacc.Bacc("TRN2", debug=False)
    x_ap = nc.dram_tensor("x", [BS, D], mybir.dt.float32, kind="ExternalInput").ap()
    out_ap = nc.dram_tensor(
        "out", [BS, K], mybir.dt.float32, kind="ExternalOutput"
    ).ap()

    lengths, offs = _segments()
    chunks = _chunks(lengths)
    n_ramp = len(RAMP_UP)
    order = _issue_order(len(chunks), n_ramp)
    assign = _engine_split(lengths, chunks, order, n_ramp)
    ramp_max = max(c[3] for c in chunks[:n_ramp])
    steady_max = max(c[3] for c in chunks[n_ramp:])

    with tile.TileContext(nc) as tc:
        with (
            tc.tile_pool(name="xp", bufs=1) as xp,
            tc.tile_pool(name="wp", bufs=1) as wp,
            tc.tile_pool(name="rp", bufs=n_ramp + 1) as rp,
            tc.tile_pool(name="op", bufs=STEADY_BUFS) as op,
        ):
            # Pre-warm the ACT activation table concurrently with the x load.
            warm = wp.tile([BS, 2], mybir.dt.float32)
            nc.vector.memset(warm[:], 0.0)
            nc.scalar.activation(
                warm[:], warm[:], mybir.ActivationFunctionType.Copy, scale=1.0
            )

            xt = xp.tile([BS, D], mybir.dt.float32)
            nc.sync.dma_start(xt[:], x_ap[:])

            for ci in order:
                s, e, off0, clen = chunks[ci]
                if ci < n_ramp:
                    ot = rp.tile([BS, ramp_max], mybir.dt.float32, tag="ramp")
                else:
                    ot = op.tile([BS, steady_max], mybir.dt.float32, tag="out")
                for i in range(s, e):
                    ln = lengths[i]
                    dst = ot[:, offs[i] - off0 : offs[i] - off0 + ln]
                    src = xt[:, i:D]
                    scal = xt[:, i : i + 1]
                    if assign[i] == "v":
                        nc.vector.tensor_scalar_mul(dst, src, scal)
                    else:
                        nc.scalar.activation(
                            dst, src, mybir.ActivationFunctionType.Copy, scale=scal
                        )
                # All output DMAs on the SP HWDGE ring: alternating across the
                # SP and ACT rings makes the SDMA engines time-slice between
                # two queues at packet granularity, lowering aggregate
                # bandwidth (A/B measured: ~190 us vs ~180 us).
                nc.sync.dma_start(out_ap[:, off0 : off0 + clen], ot[:, :clen])

    nc.compile()
    _CACHE["nc"] = nc
    return nc


def _run(x, trace=False):
    from concourse.bass_utils import run_bass_kernel_spmd

    nc = _build()
    x = np.ascontiguousarray(x, dtype=np.float32)
    assert x.shape == (B, D), x.shape
    in_maps = [{"x": x[c * BS : (c + 1) * BS]} for c in range(N_CORES)]
    res = run_bass_kernel_spmd(nc, in_maps, list(range(N_CORES)), trace=trace)
    out = np.concatenate([res.results[c]["out"] for c in range(N_CORES)], axis=0)
    return out, res


def kernel(x):
    return _run(x)[0]

